# revision 20
# baseline (speedup 1.0000x reference)
"""Llama GQA attention (B=1, S=2048, H=4096, 32 heads / 8 KV heads, RoPE, causal)
as a tensor-parallel Bass/Tile kernel on 8 Trainium2 NeuronCores.

Sharding: core c computes Q heads [4c, 4c+4) and KV head c (GQA groups align),
full causal attention for those heads, then AllGathers the (transposed)
attention outputs and computes output features [512c, 512c+512) of o_proj.

v5 (bf16):
- Sequential phases (proj all chunks -> attention chunks -> o_proj tail):
  long homogeneous matmul streams keep the PE engine queue deep so the
  LDWEIGHTS prefetch stays hidden and the HAM clock stays warm. AllGather j
  fires right after attention chunk j; all collectives hide under attention
  of later chunks and the o_proj tail.
- cos/sin precomputed host-side; causal mask via DVE multiply of a 0/1 tile
  after the exp; exp batched 2 k-blocks per ACT op ([128,2,512] PSUM pairs).
- Attention software-pipelined at DEPTH=2 (scores emitted two 2-block units
  ahead of PV), so PV never waits on the exp. Each head's normalization is
  split into two finishers flushed inside the NEXT head's matmul stream,
  so the PE never waits on a cross-engine chain:
    finA: denominator matmul (ones^T @ acc) + copy to SBUF
    finB: PE-broadcast of the row, reciprocal_approx_fast, po*rinv, DMA.
- Softmax denominator accumulated on DVE in bf16. gpsimd carries ONLY the
  AllGather triggers (a blocked gpsimd queue can't delay normalization).
"""
import numpy as np
import ml_dtypes
from contextlib import ExitStack

import concourse.bass as bass
import concourse.mybir as mybir
import concourse.tile as tile
from concourse import bacc
from concourse.bass import ts, ds
from concourse.masks import make_identity

N_CORES = 8
S = 2048
HIDDEN = 4096
NUM_HEADS = 32
HEAD_DIM = 128
HEADS_PER_CORE = NUM_HEADS // N_CORES          # 4
QSLICE = HEADS_PER_CORE * HEAD_DIM             # 512
KT = HIDDEN // 128                             # 32 contraction tiles
SC = S // 512                                  # 4 seq chunks of 512
ROPE_THETA = 10000.0

F32 = mybir.dt.float32
BF16 = mybir.dt.bfloat16

_cache = {}


def build_nc():
    nc = bacc.Bacc("TRN2", target_bir_lowering=False, debug=False,
                   num_devices=N_CORES)
    xT = nc.dram_tensor("xT", [HIDDEN, S], BF16, kind="ExternalInput").ap()
    wqT = nc.dram_tensor("wqT", [HIDDEN, QSLICE], BF16, kind="ExternalInput").ap()
    wkvT = nc.dram_tensor("wkvT", [HIDDEN, 2 * HEAD_DIM], BF16,
                          kind="ExternalInput").ap()
    woT = nc.dram_tensor("woT", [HIDDEN, QSLICE], BF16, kind="ExternalInput").ap()
    cosT = nc.dram_tensor("cosT", [HEAD_DIM, S], F32, kind="ExternalInput").ap()
    sinT = nc.dram_tensor("sinT", [HEAD_DIM, S], F32, kind="ExternalInput").ap()
    outT = nc.dram_tensor("outT", [QSLICE, S], F32, kind="ExternalOutput").ap()

    xT_r = xT.rearrange("(kt p) s -> p kt s", p=128)
    wqT_r = wqT.rearrange("(kt p) m -> p kt m", p=128)
    wkvT_r = wkvT.rearrange("(kt p) m -> p kt m", p=128)
    woT_r = woT.rearrange("(kt p) m -> p kt m", p=128)

    with tile.TileContext(nc) as tc, ExitStack() as ctx:
        const = ctx.enter_context(tc.tile_pool(name="const", bufs=1))
        bigw = ctx.enter_context(tc.tile_pool(name="bigw", bufs=1))
        slab = ctx.enter_context(tc.tile_pool(name="slab", bufs=2))
        f32t = ctx.enter_context(tc.tile_pool(name="f32t", bufs=4))
        ppool = ctx.enter_context(tc.tile_pool(name="ppool", bufs=5))
        accp = ctx.enter_context(tc.tile_pool(name="accp", bufs=2))
        smalls = ctx.enter_context(tc.tile_pool(name="smalls", bufs=2))
        rinvp = ctx.enter_context(tc.tile_pool(name="rinvp", bufs=2))
        otp = ctx.enter_context(tc.tile_pool(name="otp", bufs=2))
        dram = ctx.enter_context(tc.tile_pool(name="dram", bufs=1, space="DRAM"))
        pbig = ctx.enter_context(tc.tile_pool(name="pbig", bufs=2, space="PSUM"))
        ppo = ctx.enter_context(tc.tile_pool(name="ppo", bufs=2, space="PSUM"))
        paux = ctx.enter_context(tc.tile_pool(name="paux", bufs=2, space="PSUM"))

        # ---- persistent constants
        ones_t = const.tile([128, 1], BF16)
        nc.vector.memset(ones_t[:], 1.0)
        ones_row = const.tile([1, 128], F32)
        nc.vector.memset(ones_row[:], 1.0)
        ident = const.tile([128, 128], BF16)
        make_identity(nc, ident[:])

        # causal mask tiles: masks[k, d, q] = (q - 128d - k >= 0)
        masks_t = const.tile([128, 4, 512], BF16, name="masks_t")
        nc.gpsimd.memset(masks_t[:], 1.0)
        for d in range(4):
            nc.gpsimd.affine_select(
                masks_t[:, d, :], masks_t[:, d, :], pattern=[[1, 512]],
                compare_op=mybir.AluOpType.is_ge, fill=0.0,
                base=-128 * d, channel_multiplier=-1)

        cos_sb = const.tile([128, S], F32)
        nc.sync.dma_start(cos_sb[:], cosT[:])
        sin_sb = const.tile([128, S], F32)
        nc.sync.dma_start(sin_sb[:], sinT[:])

        qT_sb = const.tile([128, HEADS_PER_CORE, S], BF16)         # 16KB/part
        kT_sb = const.tile([128, S], BF16)                         # 4KB/part
        v_sb = const.tile([128, S // 128, HEAD_DIM], BF16)         # 4KB/part
        wkv_sb = const.tile([128, KT, 2 * HEAD_DIM], BF16)         # 16KB/part
        # wq and wo share one 32KB/part slot; wo loads once proj is done
        wq_sb = bigw.tile([128, KT, QSLICE], BF16, tag="bigw", name="wq_sb")

        ag_ins = [dram.tile([QSLICE, 512], BF16, tag=f"agin{j}",
                            name=f"agin{j}") for j in range(SC)]
        ag_outs = [dram.tile([NUM_HEADS * HEAD_DIM, 512], BF16,
                             addr_space="Shared", tag=f"agout{j}",
                             name=f"agout{j}") for j in range(SC)]

        # ---- initial DMA: x chunk 0 + kv weights first, q weights after
        x_slab0 = slab.tile([128, KT, 512], BF16, tag="slab", name="x_slab0")
        for g in range(4):  # fine-grained first quarter for a fast ramp
            kts = ds(g, 1)
            nc.sync.dma_start(x_slab0[:, kts, :], xT_r[:, kts, 0:512])
            nc.sync.dma_start(wkv_sb[:, kts, :], wkvT_r[:, kts, :])
        for g in range(1, 8):
            kts = ds(4 * g, 4)
            nc.sync.dma_start(x_slab0[:, kts, :], xT_r[:, kts, 0:512])
            nc.sync.dma_start(wkv_sb[:, kts, :], wkvT_r[:, kts, :])
        for g in range(8):
            kts = ds(4 * g, 4)
            nc.sync.dma_start(wq_sb[:, kts, :], wqT_r[:, kts, :])

        def rope(dst, src, s):
            """dst = src*cos + rotate_half(src)*sin_signed for seq chunk s."""
            rot = f32t.tile([128, 512], F32, tag="f32t", name="rot")
            nc.vector.tensor_tensor(rot[0:64, :], src[64:128, :],
                                    sin_sb[0:64, ts(s, 512)],
                                    mybir.AluOpType.mult)
            nc.vector.tensor_tensor(rot[64:128, :], src[0:64, :],
                                    sin_sb[64:128, ts(s, 512)],
                                    mybir.AluOpType.mult)
            cq = f32t.tile([128, 512], F32, tag="f32t", name="cq")
            nc.vector.tensor_tensor(cq[:], src[:], cos_sb[:, ts(s, 512)],
                                    mybir.AluOpType.mult)
            nc.vector.tensor_tensor(dst, cq[:], rot[:], mybir.AluOpType.add)

        def proj_vk(s, x_slab):
            pvk = pbig.tile([128, 2, 512], F32, tag="big", name="pvk")
            for kt in range(KT):
                nc.tensor.matmul(pvk[:, 0, :], wkv_sb[:, kt, ds(128, 128)],
                                 x_slab[:, kt, :],
                                 start=(kt == 0), stop=(kt == KT - 1))
            for kt in range(KT):
                nc.tensor.matmul(pvk[:, 1, :], wkv_sb[:, kt, ds(0, 128)],
                                 x_slab[:, kt, :],
                                 start=(kt == 0), stop=(kt == KT - 1))
            vt_c = smalls.tile([128, 512], BF16, tag="vt", name="vt_c")
            nc.scalar.copy(vt_c[:], pvk[:, 0, :])
            rope(kT_sb[:, ts(s, 512)], pvk[:, 1, :], s)
            for t in range(4):
                ptr = paux.tile([128, 128], BF16, tag="aux", name="ptr")
                nc.tensor.transpose(ptr[:], vt_c[:, ts(t, 128)], ident[:])
                nc.scalar.copy(v_sb[:, 4 * s + t, :], ptr[:])

        def proj_q(s, x_slab, hp):
            pq = pbig.tile([128, 2, 512], F32, tag="big", name="pq")
            for u in range(2):
                h = 2 * hp + u
                for kt in range(KT):
                    nc.tensor.matmul(pq[:, u, :], wq_sb[:, kt, ts(h, 128)],
                                     x_slab[:, kt, :],
                                     start=(kt == 0), stop=(kt == KT - 1))
            for u in range(2):
                h = 2 * hp + u
                rope(qT_sb[:, h, ts(s, 512)], pq[:, u, :], s)

        def attn_head(j, h, cb1, cb2):
            """Scores + exp + mask + denom-acc + PV, DEPTH=2 pipeline.
            cb1/cb2 emit the previous head's finishers inside this head's
            matmul stream. Returns (po, acc)."""
            nunit = 2 * (j + 1)
            po = ppo.tile([128, 512], F32, tag="po", name="po")
            acc = accp.tile([128, 512], BF16, tag="acc", name="acc")

            def emit_scores(u):
                ps = pbig.tile([128, 2, 512], F32, tag="big", name="ps")
                for w in range(2):
                    ki = 2 * u + w
                    nc.tensor.matmul(ps[:, w, :], kT_sb[:, ts(ki, 128)],
                                     qT_sb[:, h, ts(j, 512)],
                                     start=True, stop=True)
                pT = ppool.tile([128, 2, 512], BF16, tag="pT", name="pT")
                nc.scalar.activation(pT[:], ps[:],
                                     mybir.ActivationFunctionType.Exp)
                for w in range(2):
                    ki = 2 * u + w
                    d = ki - 4 * j
                    if d >= 0:  # diagonal block: zero the upper triangle
                        nc.vector.tensor_tensor(pT[:, w, :], pT[:, w, :],
                                                masks_t[:, d, :],
                                                mybir.AluOpType.mult)
                if u == 0:
                    nc.vector.tensor_tensor(acc[:], pT[:, 0, :], pT[:, 1, :],
                                            mybir.AluOpType.add)
                else:
                    for w in range(2):
                        nc.vector.tensor_tensor(acc[:], acc[:], pT[:, w, :],
                                                mybir.AluOpType.add)
                return pT

            DEPTH = 3
            pts = [emit_scores(uu) for uu in range(min(DEPTH, nunit))]
            if cb1 is not None:
                cb1()
            for u in range(nunit):
                pT = pts[u]
                if u + DEPTH < nunit:
                    pts.append(emit_scores(u + DEPTH))
                for w in range(2):
                    ki = 2 * u + w
                    nc.tensor.matmul(po[:], v_sb[:, ki, :], pT[:, w, :],
                                     start=(ki == 0),
                                     stop=(ki == 2 * nunit - 1))
                if u == 0 and cb2 is not None:
                    cb2()
            return po, acc

        def attn_finA(box, key):
            po, acc = box[key]
            pd = paux.tile([1, 512], F32, tag="aux", name="pd")
            nc.tensor.matmul(pd[:], ones_t[:], acc[:], start=True, stop=True)
            pd_sb = smalls.tile([1, 512], F32, tag="pd", name="pd_sb")
            nc.scalar.copy(pd_sb[:], pd[:])
            box[key] = (po, pd_sb)

        def attn_finB(j, h, box, key):
            po, pd_sb = box[key]
            rbraw = paux.tile([128, 512], F32, tag="aux", name="rbraw")
            nc.tensor.matmul(rbraw[:], ones_row[:], pd_sb[:], start=True,
                             stop=True)
            rinv = rinvp.tile([128, 512], F32, tag="rinv", name="rinv")
            nc.vector.reciprocal_approx_fast(rinv[:], rbraw[:])
            att = smalls.tile([128, 512], BF16, tag="att", name="att")
            nc.vector.tensor_tensor(att[:], po[:], rinv[:],
                                    mybir.AluOpType.mult)
            nc.sync.dma_start(ag_ins[j][ts(h, 128), :], att[:])

        def ag_trigger(j):
            nc.gpsimd.collective_compute(
                "AllGather", mybir.AluOpType.bypass,
                replica_groups=[list(range(N_CORES))],
                ins=[ag_ins[j].opt()], outs=[ag_outs[j].opt()],
            )

        a_slabs = {}

        def oproj_group(s, fp, wo_sb):
            if fp == 0:
                a_slab = slab.tile([128, KT, 512], BF16, tag="slab",
                                   name="a_slab")
                ag_r = ag_outs[s].rearrange("(kt p) s -> p kt s", p=128)
                for g in range(4):
                    kts = ds(8 * g, 8)
                    nc.sync.dma_start(a_slab[:, kts, :], ag_r[:, kts, :])
                a_slabs[s] = a_slab
            a_slab = a_slabs[s]
            pq = pbig.tile([128, 2, 512], F32, tag="big", name="pq_o")
            for u in range(2):
                ft = 2 * fp + u
                for kt in range(KT):
                    nc.tensor.matmul(pq[:, u, :], wo_sb[:, kt, ts(ft, 128)],
                                     a_slab[:, kt, :],
                                     start=(kt == 0), stop=(kt == KT - 1))
            ot = otp.tile([128, 2, 512], F32, tag="ot", name="ot")
            nc.scalar.copy(ot[:], pq[:])
            for u in range(2):
                nc.sync.dma_start(outT[ds((2 * fp + u) * 128, 128),
                                       ts(s, 512)], ot[:, u, :])

        # ---- weave: proj(0) upfront, then att(s) with proj(s+1) matmul
        # groups between heads. Attention chunk j completes early, so the
        # serialized AllGather chain (each ~35us) finishes long before the
        # o_proj tail consumes it.
        x_slabs = {0: x_slab0}

        def load_x(s):
            xs = slab.tile([128, KT, 512], BF16, tag="slab", name="x_slab")
            nc.sync.dma_start(xs[:], xT_r[:, :, ts(s, 512)])
            x_slabs[s] = xs

        load_x(1)
        proj_vk(0, x_slabs[0])
        proj_q(0, x_slabs[0], 0)
        proj_q(0, x_slabs[0], 1)

        wo_holder = {}

        def load_wo():
            # wq's last consumer (proj_q(3)) is already emitted: load wo
            wo = bigw.tile([128, KT, QSLICE], BF16, tag="bigw", name="wo_sb")
            nc.sync.dma_start(wo[:, :, 0:256], woT_r[:, :, 0:256])
            nc.sync.dma_start(wo[:, :, 256:512], woT_r[:, :, 256:512])
            wo_holder[0] = wo

        box = {}
        finq = []
        for s in range(SC):
            if s < SC - 1:
                xs = s + 1
                fillers = [
                    lambda xs=xs: (load_x(xs + 1) if xs + 1 < SC else None,
                                   proj_vk(xs, x_slabs[xs]))[-1],
                    lambda xs=xs: proj_q(xs, x_slabs[xs], 0),
                    lambda xs=xs: proj_q(xs, x_slabs[xs], 1),
                ]
                if s == SC - 2:
                    fillers.append(load_wo)
            else:
                fillers = []
            for h in range(HEADS_PER_CORE):
                cb1 = finq.pop(0) if finq else None

                def cb2(fq=list(finq)):
                    for f in fq:
                        f()
                    finq.clear()
                st = attn_head(s, h, cb1, cb2)
                key = (s, h)
                box[key] = st
                if h < len(fillers):
                    fillers[h]()
                finq.append(lambda key=key: attn_finA(box, key))
                finq.append(lambda s=s, h=h, key=key: attn_finB(s, h, box,
                                                                key))
                if h == HEADS_PER_CORE - 1:
                    finq.append(lambda s=s: ag_trigger(s))
        for f in finq:
            f()
        finq.clear()

        # ---- o_proj tail: long clean matmul stream; AllGathers done by now
        for s in range(SC):
            for fp in range(2):
                oproj_group(s, fp, wo_holder[0])

    nc.finalize()
    return nc


def _prep_inputs(hidden_states, Wq, Wk, Wv, Wo, position_ids):
    """Slice/cast per-core inputs (host-side layout prep only)."""
    bf = ml_dtypes.bfloat16
    x = np.ascontiguousarray(np.asarray(hidden_states, np.float32)[0].T).astype(bf)
    scale = 1.0 / np.sqrt(HEAD_DIM)
    # rotary tables, [head_dim, seq]; sin signed (first half negated)
    invf_half = (1.0 / (ROPE_THETA ** (np.arange(0, HEAD_DIM, 2, dtype=np.float64)
                                       / HEAD_DIM)))
    invf = np.concatenate([invf_half, invf_half])  # [128]
    pos = np.asarray(position_ids, np.float64).reshape(S)
    ang = invf[:, None] * pos[None, :]             # [128, S]
    cosT = np.cos(ang).astype(np.float32)
    sinT = np.sin(ang).astype(np.float32)
    sinT[:HEAD_DIM // 2] *= -1.0
    in_maps = []
    for c in range(N_CORES):
        wq_c = (np.asarray(Wq, np.float32)[c * QSLICE:(c + 1) * QSLICE] * scale)
        wk_c = np.asarray(Wk, np.float32)[c * HEAD_DIM:(c + 1) * HEAD_DIM]
        wv_c = np.asarray(Wv, np.float32)[c * HEAD_DIM:(c + 1) * HEAD_DIM]
        wkv_c = np.concatenate([wk_c, wv_c], axis=0)   # [256, 4096]
        wo_c = np.asarray(Wo, np.float32)[c * QSLICE:(c + 1) * QSLICE]
        in_maps.append({
            "xT": x,
            "wqT": np.ascontiguousarray(wq_c.T).astype(bf),
            "wkvT": np.ascontiguousarray(wkv_c.T).astype(bf),
            "woT": np.ascontiguousarray(wo_c.T).astype(bf),
            "cosT": cosT,
            "sinT": sinT,
        })
    return in_maps


def kernel(hidden_states, Wq, Wk, Wv, Wo, position_ids):
    from concourse.bass_utils import run_bass_kernel_spmd
    if "nc" not in _cache:
        _cache["nc"] = build_nc()
    nc = _cache["nc"]
    in_maps = _prep_inputs(hidden_states, Wq, Wk, Wv, Wo, position_ids)
    res = run_bass_kernel_spmd(nc, in_maps, core_ids=list(range(N_CORES)))
    out = np.concatenate([res.results[c]["outT"].T for c in range(N_CORES)], axis=1)
    return out[None].astype(np.float32)


# revision 24
# speedup vs baseline: 1.0496x; 1.0496x over previous
"""Llama GQA attention (B=1, S=2048, H=4096, 32 heads / 8 KV heads, RoPE, causal)
as a tensor-parallel Bass/Tile kernel on 8 Trainium2 NeuronCores.

Sharding: core c computes Q heads [4c, 4c+4) and KV head c (GQA groups align),
full causal attention for those heads, then AllGathers the (transposed)
attention outputs and computes output features [512c, 512c+512) of o_proj.

v7 (bf16):
- Emission weave: proj(0) upfront, then attention chunk s interleaved with
  proj(s+1) matmul groups between heads, o_proj as one long tail stream.
  Attention chunks finish early, so the serialized AllGather chain (~35us
  per op, one CC stream) completes long before the o_proj tail needs it,
  and the long homogeneous matmul streams keep the PE engine queue deep
  (LDWEIGHTS prefetch hidden) and the HAM clock gate warm.
- cos/sin precomputed host-side; causal mask via DVE multiply of a 0/1 tile
  after the exp; exp batched 2 k-blocks per ACT op ([128,2,512] PSUM pairs).
- Attention software-pipelined at DEPTH=3 (scores emitted three 2-block
  units ahead of PV), so PV never waits on the exp. Each head's
  normalization is split into two finishers flushed inside the NEXT head's
  matmul stream, so the PE never waits on a cross-engine chain:
    finA: denominator matmul (ones^T @ acc) + copy to SBUF
    finB: PE-broadcast of the row, reciprocal_approx_fast, po*rinv, DMA.
- Softmax denominator accumulated on DVE in bf16. gpsimd carries ONLY the
  AllGather triggers (a blocked gpsimd queue can't delay normalization).
"""
import numpy as np
import ml_dtypes
from contextlib import ExitStack

import concourse.bass as bass
import concourse.mybir as mybir
import concourse.tile as tile
from concourse import bacc
from concourse.bass import ts, ds
from concourse.masks import make_identity

N_CORES = 8
S = 2048
HIDDEN = 4096
NUM_HEADS = 32
HEAD_DIM = 128
HEADS_PER_CORE = NUM_HEADS // N_CORES          # 4
QSLICE = HEADS_PER_CORE * HEAD_DIM             # 512
KT = HIDDEN // 128                             # 32 contraction tiles
SC = S // 512                                  # 4 seq chunks of 512
ROPE_THETA = 10000.0

F32 = mybir.dt.float32
BF16 = mybir.dt.bfloat16

_cache = {}


def build_nc():
    nc = bacc.Bacc("TRN2", target_bir_lowering=False, debug=False,
                   num_devices=N_CORES)
    xT = nc.dram_tensor("xT", [HIDDEN, S], BF16, kind="ExternalInput").ap()
    wqT = nc.dram_tensor("wqT", [HIDDEN, QSLICE], BF16, kind="ExternalInput").ap()
    wkvT = nc.dram_tensor("wkvT", [HIDDEN, 2 * HEAD_DIM], BF16,
                          kind="ExternalInput").ap()
    woT = nc.dram_tensor("woT", [HIDDEN, QSLICE], BF16, kind="ExternalInput").ap()
    cosT = nc.dram_tensor("cosT", [HEAD_DIM, S], F32, kind="ExternalInput").ap()
    sinT = nc.dram_tensor("sinT", [HEAD_DIM, S], F32, kind="ExternalInput").ap()
    outT = nc.dram_tensor("outT", [QSLICE, S], F32, kind="ExternalOutput").ap()

    xT_r = xT.rearrange("(kt p) s -> p kt s", p=128)
    wqT_r = wqT.rearrange("(kt p) m -> p kt m", p=128)
    wkvT_r = wkvT.rearrange("(kt p) m -> p kt m", p=128)
    woT_r = woT.rearrange("(kt p) m -> p kt m", p=128)

    with tile.TileContext(nc) as tc, ExitStack() as ctx:
        const = ctx.enter_context(tc.tile_pool(name="const", bufs=1))
        bigw = ctx.enter_context(tc.tile_pool(name="bigw", bufs=1))
        slab = ctx.enter_context(tc.tile_pool(name="slab", bufs=2))
        f32t = ctx.enter_context(tc.tile_pool(name="f32t", bufs=4))
        ppool = ctx.enter_context(tc.tile_pool(name="ppool", bufs=5))
        accp = ctx.enter_context(tc.tile_pool(name="accp", bufs=2))
        smalls = ctx.enter_context(tc.tile_pool(name="smalls", bufs=2))
        rinvp = ctx.enter_context(tc.tile_pool(name="rinvp", bufs=2))
        otp = ctx.enter_context(tc.tile_pool(name="otp", bufs=2))
        dram = ctx.enter_context(tc.tile_pool(name="dram", bufs=1, space="DRAM"))
        pbig = ctx.enter_context(tc.tile_pool(name="pbig", bufs=2, space="PSUM"))
        ppo = ctx.enter_context(tc.tile_pool(name="ppo", bufs=2, space="PSUM"))
        paux = ctx.enter_context(tc.tile_pool(name="paux", bufs=2, space="PSUM"))

        # ---- persistent constants
        ones_t = const.tile([128, 1], BF16)
        nc.vector.memset(ones_t[:], 1.0)
        ones_row = const.tile([1, 128], F32)
        nc.vector.memset(ones_row[:], 1.0)
        ident = const.tile([128, 128], BF16)
        make_identity(nc, ident[:])

        # causal mask tiles: masks[k, d, q] = (q - 128d - k >= 0)
        masks_t = const.tile([128, 4, 512], BF16, name="masks_t")
        nc.gpsimd.memset(masks_t[:], 1.0)
        for d in range(4):
            nc.gpsimd.affine_select(
                masks_t[:, d, :], masks_t[:, d, :], pattern=[[1, 512]],
                compare_op=mybir.AluOpType.is_ge, fill=0.0,
                base=-128 * d, channel_multiplier=-1)

        cos_sb = const.tile([128, S], F32)
        nc.sync.dma_start(cos_sb[:], cosT[:])
        sin_sb = const.tile([128, S], F32)
        nc.sync.dma_start(sin_sb[:], sinT[:])

        qT_sb = const.tile([128, HEADS_PER_CORE, S], BF16)         # 16KB/part
        kT_sb = const.tile([128, S], BF16)                         # 4KB/part
        v_sb = const.tile([128, S // 128, HEAD_DIM], BF16)         # 4KB/part
        wkv_sb = const.tile([128, KT, 2 * HEAD_DIM], BF16)         # 16KB/part
        # wq and wo share one 32KB/part slot; wo loads once proj is done
        wq_sb = bigw.tile([128, KT, QSLICE], BF16, tag="bigw", name="wq_sb")

        ag_ins = [dram.tile([QSLICE, 512], BF16, tag=f"agin{j}",
                            name=f"agin{j}") for j in range(SC)]
        ag_outs = [dram.tile([NUM_HEADS * HEAD_DIM, 512], BF16,
                             addr_space="Shared", tag=f"agout{j}",
                             name=f"agout{j}") for j in range(SC)]

        # ---- initial DMA: x chunk 0 + kv weights first, q weights after
        x_slab0 = slab.tile([128, KT, 512], BF16, tag="slab", name="x_slab0")
        for g in range(4):  # fine-grained first quarter for a fast ramp
            kts = ds(g, 1)
            nc.sync.dma_start(x_slab0[:, kts, :], xT_r[:, kts, 0:512])
            nc.sync.dma_start(wkv_sb[:, kts, :], wkvT_r[:, kts, :])
        for g in range(1, 8):
            kts = ds(4 * g, 4)
            nc.sync.dma_start(x_slab0[:, kts, :], xT_r[:, kts, 0:512])
            nc.sync.dma_start(wkv_sb[:, kts, :], wkvT_r[:, kts, :])
        for g in range(8):
            kts = ds(4 * g, 4)
            nc.sync.dma_start(wq_sb[:, kts, :], wqT_r[:, kts, :])

        def rope(dst, src, s):
            """dst = src*cos + rotate_half(src)*sin_signed for seq chunk s."""
            rot = f32t.tile([128, 512], F32, tag="f32t", name="rot")
            nc.vector.tensor_tensor(rot[0:64, :], src[64:128, :],
                                    sin_sb[0:64, ts(s, 512)],
                                    mybir.AluOpType.mult)
            nc.vector.tensor_tensor(rot[64:128, :], src[0:64, :],
                                    sin_sb[64:128, ts(s, 512)],
                                    mybir.AluOpType.mult)
            cq = f32t.tile([128, 512], F32, tag="f32t", name="cq")
            nc.vector.tensor_tensor(cq[:], src[:], cos_sb[:, ts(s, 512)],
                                    mybir.AluOpType.mult)
            nc.vector.tensor_tensor(dst, cq[:], rot[:], mybir.AluOpType.add)

        def proj_vk(s, x_slab):
            # V/K interleaved per ktile: two matmuls consume each arriving
            # x ktile, so the first chunk's compute outpaces its DMA drip
            pvk = pbig.tile([128, 2, 512], F32, tag="big", name="pvk")
            for kt in range(KT):
                nc.tensor.matmul(pvk[:, 0, :], wkv_sb[:, kt, ds(128, 128)],
                                 x_slab[:, kt, :],
                                 start=(kt == 0), stop=(kt == KT - 1))
                nc.tensor.matmul(pvk[:, 1, :], wkv_sb[:, kt, ds(0, 128)],
                                 x_slab[:, kt, :],
                                 start=(kt == 0), stop=(kt == KT - 1))
            vt_c = smalls.tile([128, 512], BF16, tag="vt", name="vt_c")
            nc.scalar.copy(vt_c[:], pvk[:, 0, :])
            rope(kT_sb[:, ts(s, 512)], pvk[:, 1, :], s)
            for t in range(4):
                ptr = paux.tile([128, 128], BF16, tag="aux", name="ptr")
                nc.tensor.transpose(ptr[:], vt_c[:, ts(t, 128)], ident[:])
                nc.scalar.copy(v_sb[:, 4 * s + t, :], ptr[:])

        def proj_q(s, x_slab, hp):
            pq = pbig.tile([128, 2, 512], F32, tag="big", name="pq")
            for kt in range(KT):
                for u in range(2):
                    h = 2 * hp + u
                    nc.tensor.matmul(pq[:, u, :], wq_sb[:, kt, ts(h, 128)],
                                     x_slab[:, kt, :],
                                     start=(kt == 0), stop=(kt == KT - 1))
            for u in range(2):
                h = 2 * hp + u
                rope(qT_sb[:, h, ts(s, 512)], pq[:, u, :], s)

        def attn_head(j, h, cb1, cb2):
            """Scores + exp + mask + denom-acc + PV, DEPTH=2 pipeline.
            cb1/cb2 emit the previous head's finishers inside this head's
            matmul stream. Returns (po, acc)."""
            nunit = 2 * (j + 1)
            po = ppo.tile([128, 512], F32, tag="po", name="po")
            acc = accp.tile([128, 512], BF16, tag="acc", name="acc")

            def emit_scores(u):
                ps = pbig.tile([128, 2, 512], F32, tag="big", name="ps")
                for w in range(2):
                    ki = 2 * u + w
                    nc.tensor.matmul(ps[:, w, :], kT_sb[:, ts(ki, 128)],
                                     qT_sb[:, h, ts(j, 512)],
                                     start=True, stop=True)
                pT = ppool.tile([128, 2, 512], BF16, tag="pT", name="pT")
                nc.scalar.activation(pT[:], ps[:],
                                     mybir.ActivationFunctionType.Exp)
                for w in range(2):
                    ki = 2 * u + w
                    d = ki - 4 * j
                    if d >= 0:  # diagonal block: zero the upper triangle
                        nc.vector.tensor_tensor(pT[:, w, :], pT[:, w, :],
                                                masks_t[:, d, :],
                                                mybir.AluOpType.mult)
                if u == 0:
                    nc.vector.tensor_tensor(acc[:], pT[:, 0, :], pT[:, 1, :],
                                            mybir.AluOpType.add)
                else:
                    for w in range(2):
                        nc.vector.tensor_tensor(acc[:], acc[:], pT[:, w, :],
                                                mybir.AluOpType.add)
                return pT

            DEPTH = 3
            pts = [emit_scores(uu) for uu in range(min(DEPTH, nunit))]
            for u in range(nunit):
                pT = pts[u]
                if u + DEPTH < nunit:
                    pts.append(emit_scores(u + DEPTH))
                for w in range(2):
                    ki = 2 * u + w
                    nc.tensor.matmul(po[:], v_sb[:, ki, :], pT[:, w, :],
                                     start=(ki == 0),
                                     stop=(ki == 2 * nunit - 1))
                # previous head's finishers ride deep inside this head's
                # matmul stream so their cross-engine deps are long settled
                if u == 0 and cb1 is not None:
                    cb1()
                if u == 1 and cb2 is not None:
                    cb2()
            return po, acc

        def attn_finA(box, key):
            po, acc = box[key]
            pd = paux.tile([1, 512], F32, tag="aux", name="pd")
            nc.tensor.matmul(pd[:], ones_t[:], acc[:], start=True, stop=True)
            pd_sb = smalls.tile([1, 512], F32, tag="pd", name="pd_sb")
            nc.scalar.copy(pd_sb[:], pd[:])
            box[key] = (po, pd_sb)

        def attn_finB(j, h, box, key):
            po, pd_sb = box[key]
            rbraw = paux.tile([128, 512], F32, tag="aux", name="rbraw")
            nc.tensor.matmul(rbraw[:], ones_row[:], pd_sb[:], start=True,
                             stop=True)
            rinv = rinvp.tile([128, 512], F32, tag="rinv", name="rinv")
            nc.vector.reciprocal_approx_fast(rinv[:], rbraw[:])
            att = smalls.tile([128, 512], BF16, tag="att", name="att")
            nc.vector.tensor_tensor(att[:], po[:], rinv[:],
                                    mybir.AluOpType.mult)
            nc.sync.dma_start(ag_ins[j][ts(h, 128), :], att[:])

        def ag_trigger(j):
            nc.gpsimd.collective_compute(
                "AllGather", mybir.AluOpType.bypass,
                replica_groups=[list(range(N_CORES))],
                ins=[ag_ins[j].opt()], outs=[ag_outs[j].opt()],
            )

        a_slabs = {}

        def oproj_group(s, fp, wo_sb):
            if fp == 0:
                a_slab = slab.tile([128, KT, 512], BF16, tag="slab",
                                   name="a_slab")
                ag_r = ag_outs[s].rearrange("(kt p) s -> p kt s", p=128)
                for g in range(4):
                    kts = ds(8 * g, 8)
                    nc.sync.dma_start(a_slab[:, kts, :], ag_r[:, kts, :])
                a_slabs[s] = a_slab
            a_slab = a_slabs[s]
            pq = pbig.tile([128, 2, 512], F32, tag="big", name="pq_o")
            for u in range(2):
                ft = 2 * fp + u
                for kt in range(KT):
                    nc.tensor.matmul(pq[:, u, :], wo_sb[:, kt, ts(ft, 128)],
                                     a_slab[:, kt, :],
                                     start=(kt == 0), stop=(kt == KT - 1))
            ot = otp.tile([128, 2, 512], F32, tag="ot", name="ot")
            nc.scalar.copy(ot[:], pq[:])
            for u in range(2):
                nc.sync.dma_start(outT[ds((2 * fp + u) * 128, 128),
                                       ts(s, 512)], ot[:, u, :])

        # ---- weave: proj(0) upfront, then att(s) with proj(s+1) matmul
        # groups between heads. Attention chunk j completes early, so the
        # serialized AllGather chain (each ~35us) finishes long before the
        # o_proj tail consumes it.
        x_slabs = {0: x_slab0}

        def load_x(s):
            xs = slab.tile([128, KT, 512], BF16, tag="slab", name="x_slab")
            nc.sync.dma_start(xs[:], xT_r[:, :, ts(s, 512)])
            x_slabs[s] = xs

        load_x(1)
        proj_vk(0, x_slabs[0])
        proj_q(0, x_slabs[0], 0)
        proj_q(0, x_slabs[0], 1)

        wo_holder = {}

        def load_wo():
            # wq's last consumer (proj_q(3)) is already emitted: load wo
            wo = bigw.tile([128, KT, QSLICE], BF16, tag="bigw", name="wo_sb")
            nc.sync.dma_start(wo[:, :, 0:256], woT_r[:, :, 0:256])
            nc.sync.dma_start(wo[:, :, 256:512], woT_r[:, :, 256:512])
            wo_holder[0] = wo

        box = {}
        finq = []
        for s in range(SC):
            if s < SC - 1:
                xs = s + 1
                fillers = [
                    lambda xs=xs: (load_x(xs + 1) if xs + 1 < SC else None,
                                   proj_vk(xs, x_slabs[xs]))[-1],
                    lambda xs=xs: proj_q(xs, x_slabs[xs], 0),
                    lambda xs=xs: proj_q(xs, x_slabs[xs], 1),
                ]
                if s == SC - 2:
                    fillers.append(load_wo)
            else:
                fillers = []
            for h in range(HEADS_PER_CORE):
                cb1 = finq.pop(0) if finq else None

                def cb2(fq=list(finq)):
                    for f in fq:
                        f()
                    finq.clear()
                st = attn_head(s, h, cb1, cb2)
                key = (s, h)
                box[key] = st
                if h < len(fillers):
                    fillers[h]()
                finq.append(lambda key=key: attn_finA(box, key))
                finq.append(lambda s=s, h=h, key=key: attn_finB(s, h, box,
                                                                key))
                if h == HEADS_PER_CORE - 1:
                    finq.append(lambda s=s: ag_trigger(s))
        for f in finq:
            f()
        finq.clear()

        # ---- o_proj tail: long clean matmul stream; AllGathers done by now
        for s in range(SC):
            for fp in range(2):
                oproj_group(s, fp, wo_holder[0])

    nc.finalize()
    return nc


def _prep_inputs(hidden_states, Wq, Wk, Wv, Wo, position_ids):
    """Slice/cast per-core inputs (host-side layout prep only)."""
    bf = ml_dtypes.bfloat16
    x = np.ascontiguousarray(np.asarray(hidden_states, np.float32)[0].T).astype(bf)
    scale = 1.0 / np.sqrt(HEAD_DIM)
    # rotary tables, [head_dim, seq]; sin signed (first half negated)
    invf_half = (1.0 / (ROPE_THETA ** (np.arange(0, HEAD_DIM, 2, dtype=np.float64)
                                       / HEAD_DIM)))
    invf = np.concatenate([invf_half, invf_half])  # [128]
    pos = np.asarray(position_ids, np.float64).reshape(S)
    ang = invf[:, None] * pos[None, :]             # [128, S]
    cosT = np.cos(ang).astype(np.float32)
    sinT = np.sin(ang).astype(np.float32)
    sinT[:HEAD_DIM // 2] *= -1.0
    in_maps = []
    for c in range(N_CORES):
        wq_c = (np.asarray(Wq, np.float32)[c * QSLICE:(c + 1) * QSLICE] * scale)
        wk_c = np.asarray(Wk, np.float32)[c * HEAD_DIM:(c + 1) * HEAD_DIM]
        wv_c = np.asarray(Wv, np.float32)[c * HEAD_DIM:(c + 1) * HEAD_DIM]
        wkv_c = np.concatenate([wk_c, wv_c], axis=0)   # [256, 4096]
        wo_c = np.asarray(Wo, np.float32)[c * QSLICE:(c + 1) * QSLICE]
        in_maps.append({
            "xT": x,
            "wqT": np.ascontiguousarray(wq_c.T).astype(bf),
            "wkvT": np.ascontiguousarray(wkv_c.T).astype(bf),
            "woT": np.ascontiguousarray(wo_c.T).astype(bf),
            "cosT": cosT,
            "sinT": sinT,
        })
    return in_maps


def kernel(hidden_states, Wq, Wk, Wv, Wo, position_ids):
    from concourse.bass_utils import run_bass_kernel_spmd
    if "nc" not in _cache:
        _cache["nc"] = build_nc()
    nc = _cache["nc"]
    in_maps = _prep_inputs(hidden_states, Wq, Wk, Wv, Wo, position_ids)
    res = run_bass_kernel_spmd(nc, in_maps, core_ids=list(range(N_CORES)))
    out = np.concatenate([res.results[c]["outT"].T for c in range(N_CORES)], axis=1)
    return out[None].astype(np.float32)
